# revision 15
# baseline (speedup 1.0000x reference)
"""Trainium2 Bass kernel for nn_CompositeEmbeddingA (octree composite embedding).

Three SPMD launches on 8 NeuronCores (core = item*2 + half):

  P1: embedding gather-sum for every segment via non-transpose dma_gather
      (token-major bf16 rows) from on-device-built combined tables, xbar
      dma_start_transpose to channel-major, first-level convs W4..W8a on PE,
      conv outputs transposed back by xbar and exported bf16.  Gather index
      streams are fully host-computed (int16, phase-major column order within
      each 2048-token tile so conv matmul moving operands are contiguous).
  P2: substituted sequences s7=sub(emb7,c8), s6=sub(emb6,c7a) via one indexed
      gather each from [e_mine; c_full] row sources (indices host-computed),
      then convs W8b/W7b -> c7b/c6b.
  P3: final d6/d7/d8 rows via token-order indexed gathers, direct bf16 export.

All descriptor generation is per-row on the GpSimd Q7 pair, so the design
minimizes gathered rows and keeps everything else off that engine.  The host
only slices/concats arrays and computes gather indices (cumsum bookkeeping).

Returns the full [4, 25672, 256] f32 output (host upcast of bf16 exports).
"""

import numpy as np
import ml_dtypes

import concourse.bass as bass
import concourse.tile as tile
from concourse import bacc, mybir

BF16 = mybir.dt.bfloat16
F32 = mybir.dt.float32
I16 = mybir.dt.int16
AOP = mybir.AluOpType
ACT_IDENT = mybir.ActivationFunctionType.Identity
NPBF16 = ml_dtypes.bfloat16

P = 128
E = 256
B = 4
COUNTS = [8, 64, 256, 1024, 4096, 16384, 32768, 65536]
OFF = [0, 8, 72, 328, 1352, 5448, 21832, 54600, 120136]
NOUT = 25672
KSZ = {4: 4, 5: 8, 6: 8, 7: 8, 8: 8}
NQ = 1                # SWDGE queues: gather descriptor generation serializes on
                      # the GpSimd engine regardless, and multi-queue gathers
                      # showed intermittent data races -> single queue
TPAD = 384            # padded rows per pos-table block
T0_BLOCK = 3 * TPAD   # rows per depth in combined val+dep+pos0 table (1152)
TILE = 2048           # tokens per gather/conv tile

MS1 = {"123": 384, "4": 512, "5": 2048, "6": 8192, "7": 16384, "8": 32768}

_QCTR = [0]


def next_q():
    q = _QCTR[0] % NQ
    _QCTR[0] += 1
    return q


def seg_val(a, d):
    return a[:, OFF[d - 1]:OFF[d]]


# ===========================================================================
# device-side helpers
# ===========================================================================

def build_tables(nc, sb, psb, dram, pos_pad, val_tab, dep_tab):
    """Build bf16 gather tables in DRAM:
       t0_all [8*1152, 256]: row (d-1)*1152+(v-1)*384+p = val[v]+dep[d]+pos0[p]
       t1, t2 [384, 256]: pos_tab[1], pos_tab[2] (rows 0, 257..383 zero)."""
    t0_all = dram.tile([8 * T0_BLOCK, E], BF16)
    t1 = dram.tile([TPAD, E], BF16)
    t2 = dram.tile([TPAD, E], BF16)

    pos_sb = []
    for a in range(3):
        t = sb.tile([P, 3, E], F32, tag=f"pos_stage{a}")
        nc.sync.dma_start(t[:], pos_pad[a].rearrange("(j p) e -> p j e", p=P))
        pos_sb.append(t)

    for a, tdst in ((1, t1), (2, t2)):
        tb = sb.tile([P, 3, E], BF16, tag=f"pos_bf{a}")
        nc.vector.tensor_copy(tb[:], pos_sb[a][:])
        nc.sync.dma_start(tdst[:].rearrange("(j p) e -> p j e", p=P), tb[:])

    vt = sb.tile([1, 4 * E], F32, tag="vt")
    dt_ = sb.tile([1, 9 * E], F32, tag="dt")
    nc.sync.dma_start(vt[:], val_tab[:].rearrange("v e -> (v e)"))
    nc.sync.dma_start(dt_[:], dep_tab[:].rearrange("v e -> (v e)"))
    vd = sb.tile([1, 24 * E], F32, tag="vd")
    for d in range(1, 9):
        for v in range(1, 4):
            r = 3 * (d - 1) + (v - 1)
            nc.vector.tensor_tensor(
                vd[:, r * E:(r + 1) * E], vt[:, v * E:(v + 1) * E],
                dt_[:, d * E:(d + 1) * E], AOP.add)

    ones_f = sb.tile([1, P], F32, tag="ones_f")
    nc.vector.memset(ones_f[:], 1.0)
    for d in range(1, 9):
        stage = sb.tile([P, 9, E], BF16, tag="t0_stage")
        for v in range(1, 4):
            r = 3 * (d - 1) + (v - 1)
            bc = psb.tile([P, E], F32, tag="bc_ps")
            nc.tensor.matmul(bc[:], ones_f[:], vd[:, r * E:(r + 1) * E],
                             start=True, stop=True)
            for j in range(3):
                nc.vector.tensor_tensor(
                    stage[:, 3 * (v - 1) + j, :], pos_sb[0][:, j, :], bc[:],
                    AOP.add)
        nc.sync.dma_start(
            t0_all[(d - 1) * T0_BLOCK:d * T0_BLOCK, :].rearrange(
                "(j p) e -> p j e", p=P),
            stage[:])
    return t0_all, t1, t2


def load_w(nc, sb, w_dram, k, tag):
    """host-precast W bf16 [k,256,256] -> SBUF [128, k, 2, 2, 128]
    (p = in-ch within half j; dims: phase, in-half j, out-half m, out col)."""
    wb = sb.tile([P, k, 2, 2, P], BF16, tag=tag)
    nc.sync.dma_start(
        wb[:], w_dram[:].rearrange("k (j p) (m c) -> p k j m c", p=P, c=P))
    return wb


def load_bias(nc, sb, b_dram, tag):
    bt = sb.tile([P, 2], F32, tag=tag)
    nc.sync.dma_start(bt[:], b_dram[:].rearrange("(m p) -> p m", p=P))
    return bt


def gather3(nc, pool, tabs, ixt, c0, T, tag):
    """3 non-transpose gathers + 2 DVE adds -> token(col)-major
    [128, T/128, E] bf16 (col c*128+p at partition p, block c)."""
    g = []
    for nm, (tab, ix) in enumerate(zip(tabs, ixt)):
        o = pool.tile([P, T // P, E], BF16, tag=f"gg{nm}")
        nc.gpsimd.dma_gather(
            out_ap=o[:], in_ap=tab[:], idxs_ap=ix[:, c0:c0 + T // 16],
            num_idxs=T, num_idxs_reg=T, elem_size=E, transpose=False,
            single_packet=False, queue_num=next_q())
        g.append(o)
    f0 = g[0][:].rearrange("p c e -> p (c e)")
    nc.vector.tensor_tensor(f0, f0, g[1][:].rearrange("p c e -> p (c e)"),
                            AOP.add)
    nc.vector.tensor_tensor(f0, f0, g[2][:].rearrange("p c e -> p (c e)"),
                            AOP.add)
    return g[0]


def gather1(nc, pool, src, ix, c0, T, tag, q=None):
    o = pool.tile([P, T // P, E], BF16, tag=tag)
    nc.gpsimd.dma_gather(
        out_ap=o[:], in_ap=src[:], idxs_ap=ix[:, c0:c0 + T // 16],
        num_idxs=T, num_idxs_reg=T, elem_size=E, transpose=False,
        single_packet=False, queue_num=next_q() if q is None else q)
    return o


def xbar_tok2ch(nc, pool, g, T, tag):
    """token-major [128, T/128, E] -> ch-major [128, 2, T] bf16 via xbar
    dma transpose (one instruction per 128-token block, split sync/scalar)."""
    emb = pool.tile([P, 2, T], BF16, tag="embc")
    for c in range(T // P):
        nc.sync.dma_start_transpose(emb[:, :, c * P:(c + 1) * P], g[:, c, :])
    return emb


def conv_export(nc, pool, ps, emb, wb, bias, k, T, dram_out, g0, tag):
    """ch-major emb (phase-major cols) -> conv -> xbar transpose ->
    token(group)-major bf16 rows exported to dram_out[g0:g0+T/k]."""
    G = T // k
    co = pool.tile([P, 2, G], BF16, tag="co")
    for mo in range(2):
        pt = ps.tile([P, G], F32, tag="cps")
        i = 0
        for j in range(2):
            for ph in range(k):
                nc.tensor.matmul(pt[:], wb[:, ph, j, mo, :],
                                 emb[:, j, ph * G:(ph + 1) * G],
                                 start=(i == 0), stop=(i == 2 * k - 1))
                i += 1
        nc.scalar.activation(co[:, mo, :], pt[:], ACT_IDENT,
                             bias=bias[:, mo:mo + 1], scale=1.0)
    cot = pool.tile([P, G // P, 2, P], BF16, tag="cot")
    for mo in range(2):
        nc.sync.dma_start_transpose(cot[:, :, mo, :], co[:, mo, :])
    nc.sync.dma_start(
        dram_out[g0:g0 + G, :].rearrange("(c p) e -> p c e", p=P),
        cot[:].rearrange("p c m q -> p c (m q)"))


# ===========================================================================
# programs
# ===========================================================================

def build_p1(nc):
    pos_pad = nc.dram_tensor("pos_pad", [3, TPAD, E], F32, kind="ExternalInput")
    val_tab = nc.dram_tensor("val_tab", [4, E], F32, kind="ExternalInput")
    dep_tab = nc.dram_tensor("dep_tab", [9, E], F32, kind="ExternalInput")
    wdr, bdr = {}, {}
    for d, nm in ((4, "4"), (5, "5"), (6, "6"), (7, "7a"), (8, "8a")):
        wdr[d] = nc.dram_tensor(f"W{nm}", [KSZ[d], E, E], BF16,
                                kind="ExternalInput")
        bdr[d] = nc.dram_tensor(f"b{nm}", [E], F32, kind="ExternalInput")
    ixin = {s: nc.dram_tensor(f"ix_{s}", [3, P, m_ // 16], I16,
                              kind="ExternalInput")
            for s, m_ in MS1.items()}

    out123 = nc.dram_tensor("out123", [384, E], BF16, kind="ExternalOutput")
    e_out = {s: nc.dram_tensor(f"e{s}", [MS1[s], E], BF16,
                               kind="ExternalOutput")
             for s in ("5", "6", "7")}
    cex = {"4": ("out4", 128), "5": ("out5", 256), "6": ("c6", 1024),
           "7": ("c7a", 2048), "8": ("c8", 4096)}
    c_out = {s: nc.dram_tensor(nm, [g_, E], BF16, kind="ExternalOutput")
             for s, (nm, g_) in cex.items()}

    with tile.TileContext(nc) as tc:
        with tc.tile_pool(name="sb", bufs=1) as sb, \
             tc.tile_pool(name="gat", bufs=2) as gat, \
             tc.tile_pool(name="ps", bufs=2, space="PSUM") as ps, \
             tc.tile_pool(name="dram", bufs=1, space="DRAM") as dram:

            tabs = build_tables(nc, sb, ps, dram, pos_pad, val_tab, dep_tab)
            wts = {d: load_w(nc, sb, wdr[d], KSZ[d], f"w{d}")
                   for d in (4, 5, 6, 7, 8)}
            bias = {d: load_bias(nc, sb, bdr[d], f"b{d}")
                    for d in (4, 5, 6, 7, 8)}
            ixt = {}
            for s, m_ in MS1.items():
                t3 = []
                for a in range(3):
                    t = sb.tile([P, m_ // 16], I16, tag=f"ix{s}{a}")
                    nc.sync.dma_start(t[:], ixin[s][a])
                    t3.append(t)
                ixt[s] = t3

            g = gather3(nc, gat, tabs, ixt["123"], 0, 384, "s123")
            nc.sync.dma_start(out123[:].rearrange("(c p) e -> p c e", p=P),
                              g[:])

            for s in ("4", "5", "6", "7", "8"):
                m_, k = MS1[s], KSZ[int(s)]
                d = int(s)
                for t0 in range(0, m_, TILE):
                    T = min(TILE, m_)
                    g = gather3(nc, gat, tabs, ixt[s], t0 // 16, T, f"s{s}")
                    if s in e_out:
                        nc.sync.dma_start(
                            e_out[s][t0:t0 + T, :].rearrange(
                                "(c p) e -> p c e", p=P),
                            g[:])
                    emb = xbar_tok2ch(nc, gat, g, T, f"x{s}")
                    conv_export(nc, gat, ps, emb, wts[d], bias[d], k, T,
                                c_out[s], t0 // k, f"c{s}")
    nc.compile()


def build_p2(nc):
    src7 = nc.dram_tensor("src7", [16384 + 8192, E], BF16, kind="ExternalInput")
    src6 = nc.dram_tensor("src6", [8192 + 4096, E], BF16, kind="ExternalInput")
    ix7d = nc.dram_tensor("ix7", [P, 1024], I16, kind="ExternalInput")
    ix6d = nc.dram_tensor("ix6", [P, 512], I16, kind="ExternalInput")
    w8b = nc.dram_tensor("W8b", [8, E, E], BF16, kind="ExternalInput")
    b8b = nc.dram_tensor("b8b", [E], F32, kind="ExternalInput")
    w7b = nc.dram_tensor("W7b", [8, E, E], BF16, kind="ExternalInput")
    b7b = nc.dram_tensor("b7b", [E], F32, kind="ExternalInput")

    c7b = nc.dram_tensor("c7b", [2048, E], BF16, kind="ExternalOutput")
    c6b = nc.dram_tensor("c6b", [1024, E], BF16, kind="ExternalOutput")

    with tile.TileContext(nc) as tc:
        with tc.tile_pool(name="sb", bufs=1) as sb, \
             tc.tile_pool(name="gat", bufs=3) as gat, \
             tc.tile_pool(name="ps", bufs=2, space="PSUM") as ps:
            for (nm, src, ixd, m_, wd, bd, outd) in (
                    ("7", src7, ix7d, 16384, w8b, b8b, c7b),
                    ("6", src6, ix6d, 8192, w7b, b7b, c6b)):
                wb = load_w(nc, sb, wd, 8, f"w{nm}")
                bias = load_bias(nc, sb, bd, f"bb{nm}")
                ix = sb.tile([P, m_ // 16], I16, tag=f"ix{nm}")
                nc.sync.dma_start(ix[:], ixd[:])
                for t0 in range(0, m_, TILE):
                    g = gather1(nc, gat, src, ix, t0 // 16, TILE, "sg")
                    emb = xbar_tok2ch(nc, gat, g, TILE, f"x{nm}")
                    conv_export(nc, gat, ps, emb, wb, bias, 8, TILE,
                                outd, t0 // 8, f"c{nm}")
    nc.compile()


def build_p3(nc):
    src_d6 = nc.dram_tensor("src_d6", [2048 + 2048, E], BF16,
                            kind="ExternalInput")
    src_d7 = nc.dram_tensor("src_d7", [2048 + 2048, E], BF16,
                            kind="ExternalInput")
    src_d8 = nc.dram_tensor("src_d8", [8192 + 4096, E], BF16,
                            kind="ExternalInput")
    ix6d = nc.dram_tensor("ix_o6", [P, 128], I16, kind="ExternalInput")
    ix8d = nc.dram_tensor("ix_o8", [P, 512], I16, kind="ExternalInput")

    o6 = nc.dram_tensor("o6", [2048, E], BF16, kind="ExternalOutput")
    o7 = nc.dram_tensor("o7", [2048, E], BF16, kind="ExternalOutput")
    o8 = nc.dram_tensor("o8", [8192, E], BF16, kind="ExternalOutput")

    with tile.TileContext(nc) as tc:
        with tc.tile_pool(name="sb", bufs=1) as sb, \
             tc.tile_pool(name="gat", bufs=3) as gat:
            ix6 = sb.tile([P, 128], I16, tag="ix6")
            nc.sync.dma_start(ix6[:], ix6d[:])
            ix8 = sb.tile([P, 512], I16, tag="ix8")
            nc.sync.dma_start(ix8[:], ix8d[:])
            for (nm, src, ix, m_, outd) in (
                    ("6", src_d6, ix6, 2048, o6),
                    ("7", src_d7, ix6, 2048, o7),
                    ("8", src_d8, ix8, 8192, o8)):
                for t0 in range(0, m_, TILE):
                    g = gather1(nc, gat, src, ix, t0 // 16, TILE, f"g{nm}",
                                q=0)
                    nc.sync.dma_start(
                        outd[t0:t0 + TILE, :].rearrange("(c p) e -> p c e",
                                                        p=P),
                        g[:])
    nc.compile()


# ===========================================================================
# host orchestration
# ===========================================================================

_PROGRAMS = {}
LAST_RESULTS = []   # BassKernelResults of the launches of the last kernel()


def _get_program(name, builder):
    if name not in _PROGRAMS:
        nc = bacc.Bacc("TRN2", target_bir_lowering=False, debug=False,
                       num_swdge_queues=NQ)
        builder(nc)
        _PROGRAMS[name] = nc
    return _PROGRAMS[name]


def _run(nc, in_maps, **kw):
    from concourse import bass_utils
    res = bass_utils.run_bass_kernel_spmd(
        nc, in_maps, core_ids=list(range(len(in_maps))), **kw)
    LAST_RESULTS.append(res)
    return res


def _wrap16(a):
    """[m] -> [16, m/16] i16 (token t at [t%16, t//16])."""
    a = np.asarray(a, np.int16)
    return np.ascontiguousarray(a.reshape(-1, 16).T)


def _rep(a):
    """[16, w] -> [128, w] (replicated for all SWDGE queue cpu pairs)."""
    return np.tile(a, (8, 1))


def _colperm(m, k, T):
    """col position -> token index (phase-major within each T-token tile)."""
    G = T // k
    cols = np.arange(T)
    tloc = (cols % G) * k + cols // G
    base = (np.arange(m // T) * T)[:, None]
    return (base + tloc[None, :]).reshape(-1)


def _rowof(m, k, T):
    """token index -> export row (inverse of _colperm)."""
    p = _colperm(m, k, T)
    inv = np.empty(m, np.int64)
    inv[p] = np.arange(m)
    return inv


_ROWOF = {5: _rowof(2048, 8, 2048), 6: _rowof(8192, 8, 2048),
          7: _rowof(16384, 8, 2048)}


def _ix3(i0, i1, i2):
    return np.stack([_rep(_wrap16(i0)), _rep(_wrap16(i1)),
                     _rep(_wrap16(i2))])


def make_p1_inputs(value, position, weights):
    pos_tab = np.asarray(weights["pos_tab"], np.float32)
    pos_pad = np.zeros((3, TPAD, E), np.float32)
    pos_pad[:, :257, :] = pos_tab
    shared = dict(
        pos_pad=pos_pad,
        val_tab=np.asarray(weights["val_tab"], np.float32),
        dep_tab=np.asarray(weights["dep_tab"], np.float32),
    )
    for d, nm in ((4, "4"), (5, "5"), (6, "6"), (7, "7a"), (8, "8a")):
        shared[f"W{nm}"] = np.asarray(weights[f"W{nm}"], np.float32).astype(NPBF16)
        shared[f"b{nm}"] = np.asarray(weights[f"b{nm}"], np.float32)

    in_maps = []
    for i in range(B):
        for h in range(2):
            m = dict(shared)
            # seg 1-3: pad each half to 128 tokens (value=1, pos=1)
            i0 = np.zeros(384, np.int64)
            i1 = np.ones(384, np.int64)
            i2 = np.ones(384, np.int64)
            for d in (1, 2, 3):
                n = COUNTS[d - 1] // 2
                v = seg_val(value, d)[i, h * n:(h + 1) * n]
                po = position[i, OFF[d - 1] + h * n:OFF[d - 1] + (h + 1) * n]
                blk = slice((d - 1) * 128, (d - 1) * 128 + 128)
                vv = np.ones(128, np.int64)
                pp = np.ones((128, 3), np.int64)
                vv[:n] = v
                pp[:n] = po
                i0[blk] = (d - 1) * T0_BLOCK + (vv - 1) * TPAD + pp[:, 0]
                i1[blk] = pp[:, 1]
                i2[blk] = pp[:, 2]
            m["ix_123"] = _ix3(i0, i1, i2)
            for d in range(4, 9):
                n = COUNTS[d - 1] // 2
                k = KSZ[d]
                perm = _colperm(n, k, min(TILE, n))
                v = seg_val(value, d)[i, h * n:(h + 1) * n][perm]
                po = position[i, OFF[d - 1] + h * n:OFF[d - 1] + (h + 1) * n][perm]
                i0 = (d - 1) * T0_BLOCK + (v - 1) * TPAD + po[:, 0]
                m[f"ix_{d}"] = _ix3(i0, po[:, 1], po[:, 2])
            in_maps.append(m)
    return in_maps


def _sub_idx(val_full, h, m, rowof, base, permute):
    """Column-order gather idx for substituted stream over half h.

    val_full: [2m] parent segment values; rowof: local token -> e-row;
    base: first child row in the gather source; child rank = global cumsum."""
    mask = val_full == 2
    crank = np.cumsum(mask)
    t = np.arange(m) + h * m
    idx = np.where(mask[t], base + crank[t] - 1, rowof[:m])
    if permute:
        idx = idx[_colperm(m, 8, TILE)]
    return _rep(_wrap16(idx))


def kernel(**inputs):
    value = np.asarray(inputs["value"])
    position = np.asarray(inputs["position"])
    weights = {k: np.asarray(v) for k, v in inputs.items()
               if k not in ("value", "depth", "position")}

    LAST_RESULTS.clear()

    # ---------------- P1 ----------------
    nc1 = _get_program("p1", build_p1)
    r1 = _run(nc1, make_p1_inputs(value, position, weights)).results

    # ---------------- P2 ----------------
    nc2 = _get_program("p2", build_p2)
    in2 = []
    for i in range(B):
        c8_full = np.concatenate([r1[2 * i]["c8"], r1[2 * i + 1]["c8"]])
        c7a_full = np.concatenate([r1[2 * i]["c7a"], r1[2 * i + 1]["c7a"]])
        for h in range(2):
            in2.append(dict(
                src7=np.concatenate([r1[2 * i + h]["e7"], c8_full]),
                src6=np.concatenate([r1[2 * i + h]["e6"], c7a_full]),
                ix7=_sub_idx(np.asarray(seg_val(value, 7)[i]), h, 16384,
                             _ROWOF[7], 16384, True),
                ix6=_sub_idx(np.asarray(seg_val(value, 6)[i]), h, 8192,
                             _ROWOF[6], 8192, True),
                W8b=np.asarray(weights["W8b"], np.float32).astype(NPBF16),
                b8b=np.asarray(weights["b8b"], np.float32),
                W7b=np.asarray(weights["W7b"], np.float32).astype(NPBF16),
                b7b=np.asarray(weights["b7b"], np.float32),
            ))
    r2 = _run(nc2, in2).results

    # ---------------- P3 ----------------
    nc3 = _get_program("p3", build_p3)
    in3 = []
    for i in range(B):
        c6_full = np.concatenate([r1[2 * i]["c6"], r1[2 * i + 1]["c6"]])
        c6b_full = np.concatenate([r2[2 * i]["c6b"], r2[2 * i + 1]["c6b"]])
        c7b_full = np.concatenate([r2[2 * i]["c7b"], r2[2 * i + 1]["c7b"]])
        v5 = np.asarray(seg_val(value, 5)[i])
        v6 = np.asarray(seg_val(value, 6)[i])
        for h in range(2):
            in3.append(dict(
                src_d6=np.concatenate([r1[2 * i + h]["e5"], c6_full]),
                src_d7=np.concatenate([r1[2 * i + h]["e5"], c6b_full]),
                src_d8=np.concatenate([r1[2 * i + h]["e6"], c7b_full]),
                ix_o6=_sub_idx(v5, h, 2048, _ROWOF[5], 2048, False),
                ix_o8=_sub_idx(v6, h, 8192, _ROWOF[6], 8192, False),
            ))
    r3 = _run(nc3, in3).results

    # ---------------- assemble ----------------
    out = np.zeros((B, NOUT, E), np.float32)
    for i in range(B):
        pieces = []
        for d, valid in ((1, 4), (2, 32), (3, 128)):
            for h in range(2):
                blk = r1[2 * i + h]["out123"][(d - 1) * 128:
                                              (d - 1) * 128 + valid]
                pieces.append(blk)
        for h in range(2):
            pieces.append(r1[2 * i + h]["out4"])
        for h in range(2):
            pieces.append(r1[2 * i + h]["out5"])
        for h in range(2):
            pieces.append(r3[2 * i + h]["o6"])
        for h in range(2):
            pieces.append(r3[2 * i + h]["o7"])
        for h in range(2):
            pieces.append(r3[2 * i + h]["o8"])
        out[i] = np.concatenate(pieces, axis=0).astype(np.float32)
    return out
